# revision 1
# baseline (speedup 1.0000x reference)
"""CWT (Morlet wavelet transform) + per-sample min-max norm + bilinear resize
to (200, 200), as a Bass/Tile kernel for 8 Trainium2 NeuronCores.

Math (verified vs the jax reference to ~8e-6 abs):
  res[b, s, w] = sum_m x[b, m] * K[s, w + 512 - m]          (conv, SAME)
  out[b]       = (Rh @ res[b] @ Rw.T - mn_b) / (mx_b - mn_b)
with mn/mx over the full-res res[b], and Rh/Rw the half-pixel bilinear
resize matrices (resize commutes with the per-sample affine norm).

Device strategy (pure data parallel, 16 samples/core):
  - Host pads x to xph[b, 2048] (512 zeros each side).
  - A single strided DMA per sample builds the forward Toeplitz "strip"
    strip[j, u] = xph[b, u + 1 + j]  (128 x 1920); with host-row-reversed
    kernel chunks, every contraction chunk of the conv is a plain slice:
      res[b, :, w0:w0+512] += KR_c.T @ strip[:, w0 + 896 - 128c :][:512]
    TensorE fp32r (full-rate fp32), PSUM accumulation over the 8 chunks.
  - PSUM->SBUF copies on ScalarE; per-partition min(negated)/max on
    VectorE; cross-partition finish via one PE transpose + reduce, and
    the per-sample scalars are partition-broadcast with a rank-1 matmul.
    The finish runs once per half so the first half's tail overlaps the
    second half's conv.
  - W-resize (1024->200) exploits exact 128-periodicity of the bilinear
    taps (25 outputs per 128 inputs): 25 x (2 scaled muls + 1 add), VectorE.
  - H-resize (101->200) is a single matmul per output tile (contraction
    101 <= 128); normalization fused into the PSUM->SBUF copy on ScalarE.
"""

from contextlib import ExitStack

import numpy as np

import concourse.bacc as bacc
import concourse.bass as bass
import concourse.tile as tile
from concourse import mybir
from concourse.bass_utils import run_bass_kernel_spmd

B, N, S = 128, 1024, 101
NCORES = 8
BP = B // NCORES  # samples per core
OH = OW = 200
PER = 25  # resize outputs per 128-column period (25 * 8 = 200)
NPERIOD = 8
STRIP_W = 1920
XPH_W = 2048

F32 = mybir.dt.float32
F32R = mybir.dt.float32r
F16 = mybir.dt.float16

# sentinel for unwritten P columns: big enough to be max-neutral, small
# enough that doubling it stays finite in fp32 (the half-0 finish pass
# computes garbage-but-finite scalars for half-1 columns)
_FBIG = 1.0e30


def _lin_taps(n_in, n_out):
    src = (np.arange(n_out, dtype=np.float64) + 0.5) * (n_in / n_out) - 0.5
    w0 = np.floor(src).astype(np.int64)
    f = src - w0
    return w0, f


_WH0, _FH = _lin_taps(S, OH)
_WW0, _FW = _lin_taps(N, OW)
# exact periodicity: w0[j + 25] == w0[j] + 128
assert all(_WW0[j + PER] == _WW0[j] + 128 for j in range(OW - PER))
O_J = [int(v) for v in _WW0[:PER]]
A_J = [float(1.0 - f) for f in _FW[:PER]]
B_J = [float(f) for f in _FW[:PER]]
assert min(O_J) >= 0 and max(O_J) + 1 < 128


def _build_rhT():
    Rh = np.zeros((OH, S), np.float64)
    for i in range(OH):
        w0, f = int(_WH0[i]), float(_FH[i])
        Rh[i, min(max(w0, 0), S - 1)] += 1.0 - f
        Rh[i, min(max(w0 + 1, 0), S - 1)] += f
    return np.ascontiguousarray(Rh.T.astype(np.float32))  # (101, 200)


def build_nc():
    nc = bacc.Bacc(trn_type="TRN2")

    xph = nc.dram_tensor("xph", [BP, XPH_W], F16, kind="ExternalInput").ap()
    ktc = nc.dram_tensor("ktc", [128, 8, S], F16, kind="ExternalInput").ap()
    rht = nc.dram_tensor("rht", [S, OH], F32R, kind="ExternalInput").ap()
    eye = nc.dram_tensor("eye", [128, 128], F32, kind="ExternalInput").ap()
    out = nc.dram_tensor("out", [BP, OH, OW], F32, kind="ExternalOutput").ap()

    with tile.TileContext(nc) as tc, ExitStack() as ctx:
        consts = ctx.enter_context(tc.tile_pool(name="consts", bufs=1))
        strips = ctx.enter_context(tc.tile_pool(name="strips", bufs=4))
        big = ctx.enter_context(tc.tile_pool(name="big", bufs=1))
        scratch = ctx.enter_context(tc.tile_pool(name="scratch", bufs=4))
        halfs = ctx.enter_context(tc.tile_pool(name="halfs", bufs=3))
        psum_r = ctx.enter_context(tc.tile_pool(name="psum_r", bufs=2, space="PSUM"))
        psum_h = ctx.enter_context(tc.tile_pool(name="psum_h", bufs=3, space="PSUM"))
        psum_t = ctx.enter_context(tc.tile_pool(name="psum_t", bufs=1, space="PSUM"))

        def load_strip(b):
            st = strips.tile([128, STRIP_W], F16, tag="strip")
            src_ = bass.AP(
                tensor=xph.tensor,
                offset=b * XPH_W + 1,
                ap=[[1, 128], [1, STRIP_W]],
            )
            nc.sync.dma_start(out=st, in_=src_)
            return st

        # first strip, then the kernel bank: the first matmul needs exactly
        # these two, everything else can trail
        strip_handles = {0: load_strip(0)}
        kt_sb = consts.tile([128, 8, S], F16)  # [j, c, s]
        nc.sync.dma_start(out=kt_sb, in_=ktc)
        rh_sb = consts.tile([S, OH], F32R)
        nc.sync.dma_start(out=rh_sb, in_=rht)
        eye_sb = consts.tile([128, 128], F32)
        nc.sync.dma_start(out=eye_sb, in_=eye)
        ones1 = consts.tile([1, 128], F32)
        nc.vector.memset(ones1, 1.0)
        # P[:, b] = -mn_b, P[:, BP+b] = mx_b; pad rows stay -BIG (max-neutral)
        P = consts.tile([128, 2 * BP], F32)
        nc.vector.memset(P, -_FBIG)

        # --- persistent big tiles ---
        res_sb = big.tile([S, BP * N], F32)  # conv result, fp32
        out_w = big.tile([S, BP * OW], F32R)  # after W-resize
        stage0 = big.tile([100, BP * OW], F32)  # final rows 0..99
        stage1 = big.tile([100, BP * OW], F32)  # final rows 100..199
        stages = [stage0, stage1]

        res_v = res_sb.rearrange("s (b p q) -> s b p q", b=BP, q=128)
        out_w_v = out_w.rearrange("s (b p j) -> s b p j", b=BP, j=PER)

        def do_sample(b):
            st = strip_handles.pop(b) if b in strip_handles else load_strip(b)

            halves = []
            for h in range(2):
                r = psum_r.tile([S, 512], F32, tag=f"r{h}")
                w0 = h * 512
                for c in range(8):
                    off = w0 + 896 - 128 * c
                    nc.tensor.matmul(
                        r,
                        kt_sb[:, c, :],
                        st[:, off : off + 512],
                        start=(c == 0),
                        stop=(c == 7),
                    )
                halves.append(r)

            for h in range(2):
                nc.scalar.copy(
                    out=res_sb[:, b * N + h * 512 : b * N + h * 512 + 512],
                    in_=halves[h],
                )
            nc.vector.tensor_reduce(
                out=P[0:S, b : b + 1],
                in_=res_sb[:, b * N : (b + 1) * N],
                axis=mybir.AxisListType.X,
                op=mybir.AluOpType.min,
                negate=True,
            )
            nc.vector.tensor_reduce(
                out=P[0:S, BP + b : BP + b + 1],
                in_=res_sb[:, b * N : (b + 1) * N],
                axis=mybir.AxisListType.X,
                op=mybir.AluOpType.max,
            )

        def do_wresize(b0, nb):
            # 25 taps, batched over samples [b0, b0+nb) x 8 periods
            for jp in range(PER):
                u = scratch.tile([S, 8, NPERIOD], F32, tag="u")
                v = scratch.tile([S, 8, NPERIOD], F32, tag="v")
                nc.vector.tensor_scalar_mul(
                    u[:, 0:nb, :], res_v[:, b0 : b0 + nb, :, O_J[jp]], A_J[jp]
                )
                nc.vector.tensor_scalar_mul(
                    v[:, 0:nb, :], res_v[:, b0 : b0 + nb, :, O_J[jp] + 1], B_J[jp]
                )
                nc.vector.tensor_tensor(
                    out=out_w_v[:, b0 : b0 + nb, :, jp],
                    in0=u[:, 0:nb, :],
                    in1=v[:, 0:nb, :],
                    op=mybir.AluOpType.add,
                )

        def finish_minmax():
            # tp[c, s] = P[s, c]  (PE transpose), then free-dim reduce gives
            # the cross-partition max per column; broadcast back via rank-1.
            # Runs per half; the second run sees the complete P.
            tp = psum_t.tile([2 * BP, 128], F32, tag="tail")
            nc.tensor.transpose(tp, P, eye_sb)
            m2 = halfs.tile([2 * BP, 1], F32, tag="m2")
            nc.vector.tensor_reduce(
                out=m2, in_=tp, axis=mybir.AxisListType.X, op=mybir.AluOpType.max
            )
            rrow_ps = psum_t.tile([1, 2 * BP], F32, tag="tail")
            nc.tensor.transpose(rrow_ps, m2, eye_sb[0 : 2 * BP, 0 : 2 * BP])
            rrow = halfs.tile([1, 2 * BP], F32, tag="rrow")
            nc.scalar.copy(out=rrow, in_=rrow_ps)
            bc_ps = psum_t.tile([128, 2 * BP], F32, tag="tail")
            nc.tensor.matmul(bc_ps, ones1, rrow, start=True, stop=True)
            BC = halfs.tile([128, 2 * BP], F32, tag="BC")
            nc.scalar.copy(out=BC, in_=bc_ps)
            RNG = halfs.tile([128, BP], F32, tag="RNG")
            nc.vector.tensor_tensor(
                out=RNG,
                in0=BC[:, BP : 2 * BP],
                in1=BC[:, 0:BP],
                op=mybir.AluOpType.add,
            )
            scl = halfs.tile([128, BP], F32, tag="SCL")
            tb = halfs.tile([128, BP], F32, tag="TB")
            nc.vector.reciprocal(scl, RNG)
            nc.vector.tensor_tensor(
                out=tb, in0=BC[:, 0:BP], in1=scl, op=mybir.AluOpType.mult
            )
            return scl, tb

        def ship(pair, ic):
            nc.sync.dma_start(
                out=out[
                    pair * 2 : (pair + 1) * 2, ic * 100 : (ic + 1) * 100, :
                ].rearrange("b p j -> p b j"),
                in_=stages[ic][:, pair * 2 * OW : (pair + 1) * 2 * OW],
            )

        def do_hresize(chunk, scl, tb):
            # chunk covers samples (2*chunk, 2*chunk+1): 400 columns
            for ic in range(2):
                ph = psum_h.tile([100, 400], F32, tag="ph")
                nc.tensor.matmul(
                    ph,
                    rh_sb[:, ic * 100 : ic * 100 + 100],
                    out_w[:, chunk * 400 : (chunk + 1) * 400],
                    start=True,
                    stop=True,
                )
                for k in range(2):
                    b = chunk * 2 + k
                    nc.scalar.activation(
                        out=stages[ic][:, b * OW : (b + 1) * OW],
                        in_=ph[:, k * OW : (k + 1) * OW],
                        func=mybir.ActivationFunctionType.Identity,
                        bias=tb[0:100, b : b + 1],
                        scale=scl[0:100, b : b + 1],
                    )

        for h in range(2):
            b0 = h * 8
            # first 6 samples, then their whole back end; the last pair's
            # back end is all that trails the half's conv
            for b in range(b0, b0 + 6):
                do_sample(b)
            scl, tb = finish_minmax()  # valid for samples <= b0+5
            do_wresize(b0, 6)
            for pair in range(4 * h, 4 * h + 3):
                do_hresize(pair, scl, tb)
                for ic in range(2):
                    ship(pair, ic)
            for b in range(b0 + 6, b0 + 8):
                do_sample(b)
            scl, tb = finish_minmax()
            do_wresize(b0 + 6, 2)
            do_hresize(4 * h + 3, scl, tb)
            for ic in range(2):
                ship(4 * h + 3, ic)

    nc.compile()
    return nc


_CACHE = {}


def _get_nc():
    if "nc" not in _CACHE:
        _CACHE["nc"] = build_nc()
    return _CACHE["nc"]


def _host_inputs(x, kernels):
    x = np.ascontiguousarray(np.asarray(x, dtype=np.float32))
    K = np.ascontiguousarray(np.asarray(kernels, dtype=np.float32))
    assert x.shape == (B, N) and K.shape == (S, N)
    xph = np.zeros((B, XPH_W), np.float16)
    xph[:, 512 : 512 + N] = x.astype(np.float16)
    # ktc[c, j*, s] = K[s, 128c + 127 - j*]  (row-reversed chunks; pairs with
    # the forward-shifted Toeplitz strip)
    ktc = np.ascontiguousarray(K.reshape(S, 8, 128)[:, :, ::-1].transpose(2, 1, 0).astype(np.float16))
    rht = _build_rhT()
    eye = np.eye(128, dtype=np.float32)
    in_maps = [
        {
            "xph": np.ascontiguousarray(xph[c * BP : (c + 1) * BP]),
            "ktc": ktc,
            "rht": rht,
            "eye": eye,
        }
        for c in range(NCORES)
    ]
    return in_maps


def _ensure_ntff_hook_importable():
    """run_bass_kernel_spmd(trace=True) under axon imports antenv.axon_hooks,
    which some agent images lack; degrade to no-trace instead of crashing."""
    import sys
    import types

    try:
        import antenv.axon_hooks  # noqa: F401
    except ImportError:
        try:
            import antenv
        except ImportError:
            return
        mod = types.ModuleType("antenv.axon_hooks")
        mod._hook = None
        mod.get_axon_ntff_profile_hook = lambda: mod._hook
        mod.set_axon_ntff_profile_hook = lambda h: setattr(mod, "_hook", h)
        sys.modules["antenv.axon_hooks"] = mod
        antenv.axon_hooks = mod


def run_kernel_full(x, kernels, trace=False, **kwargs):
    _ensure_ntff_hook_importable()
    nc = _get_nc()
    in_maps = _host_inputs(x, kernels)
    res = run_bass_kernel_spmd(
        nc, in_maps, core_ids=list(range(NCORES)), trace=trace, **kwargs
    )
    outs = [res.results[c]["out"] for c in range(NCORES)]
    full = np.concatenate(outs, axis=0).reshape(B, OH, OW, 1)
    return np.ascontiguousarray(full.astype(np.float32)), res


def kernel(x, kernels):
    return run_kernel_full(x, kernels)[0]

